# revision 17
# baseline (speedup 1.0000x reference)
"""Trainium2 Bass kernel for fused attention layer (QKV proj + QKNorm + RoPE +
causal attention + out proj), tensor-parallel across (batch, head-group) on 8
NeuronCores.

Reference semantics (B=2, L=2048, D=2048, H=16, HD=128):
    qkv = x @ w_qkv + b_qkv ; split q,k,v per head
    q,k = rms_norm(q)*q_scale, rms_norm(k)*k_scale   (over head_dim)
    q,k = rope(q), rope(k)                           (rotate-half)
    attn = softmax(mask(q k^T / sqrt(HD)))
    out = (attn @ v) reshaped @ w_out + b_out
Sharding: core c -> batch b=c//4, heads 4g..4g+3 with g=c%4. Each core emits a
partial out-projection [L, D]; the host sums the 4 partials per batch + b_out.

v2 scheduling notes:
  - qkv bias is applied by a 17th 1-partition matmul in each accumulation
    group (stationary = bias row, moving = a [1,512] ones vector), so the
    norm/rope pipeline reads the biased projection directly from PSUM.
  - q_scale/k_scale are folded into per-q/k cos/sin tables on the host
    (rope output dim i scales by s_i, which multiplies cos_i and the sin
    factor sourced from partition (i+64)%128).
  - DMA issue order puts the first-consumed bytes (wv chunk 0, x tiles)
    ahead of wqk/constants so the PE starts ~3us in.
  - w_out streams into the freed wv/wqk slots right after the last
    projection reads, chunked so the first out-proj matmuls start early.
  - out projection DMAs straight from PSUM (no staging copy).
"""

import numpy as np

import concourse.bass as bass
import concourse.bacc as bacc
import concourse.tile as tile
import concourse.mybir as mybir
from concourse.bass_utils import run_bass_kernel_spmd

# This kernel only uses Square/Ln/Exp/Copy/Identity activations, all present
# in the 'natural_log_exp_and_others' set. The greedy per-function set
# chooser would otherwise ping-pong between 'exp_and_others' and
# 'natural_log' (a ~1.3us table load per switch, on the softmax critical
# path). Restrict candidacy to the one set that covers everything; entry
# order (= act_func_set_id) is preserved so walrus still loads the right
# table.
_orig_get_act_tables = bacc.get_activation_tables


def _act_tables_one_set(arch):
    tables = _orig_get_act_tables(arch)
    keep = "natural_log_exp_and_others"
    if keep in tables:
        tables = {
            name: (funcs if name == keep else type(funcs)())
            for name, funcs in tables.items()
        }
    return tables


bacc.get_activation_tables = _act_tables_one_set

f32 = mybir.dt.float32
f16 = mybir.dt.float16
f32r = mybir.dt.float32r
bf16 = mybir.dt.bfloat16
AF = mybir.ActivationFunctionType
ALU = mybir.AluOpType

B = 2
D = 2048
H = 16
HD = 128
KC = D // 128          # 16 contraction chunks of 128
EPS = 1e-6
ROPE_THETA = 10000.0
NEG = -1e30
N_CORES = 8
HEADS_PER_CORE = 4     # 2 pairs of 2


def build_nc(L, share_cos=True):
    TQ = L // 512       # 512-wide t chunks (query chunks, proj chunks)
    TK = L // 128       # 128-wide t chunks (key chunks, v chunks, out-proj chunks)

    nc = bacc.Bacc(None, target_bir_lowering=False)

    xT_d = nc.dram_tensor("xT", [128, KC, L], f32r, kind="ExternalInput")
    wqk_d = nc.dram_tensor("wqk", [2, 128, KC, 512], f32r, kind="ExternalInput")
    wv_d = nc.dram_tensor("wv", [2, 128, KC, 256], f32r, kind="ExternalInput")
    wout_d = nc.dram_tensor("wout", [128, 4, D], f32r, kind="ExternalInput")
    qcos_d = nc.dram_tensor("qcosT", [128, L], f32, kind="ExternalInput")
    qsin_d = nc.dram_tensor("qsinT", [128, L], f32, kind="ExternalInput")
    if not share_cos:
        kcos_d = nc.dram_tensor("kcosT", [128, L], f32, kind="ExternalInput")
        ksin_d = nc.dram_tensor("ksinT", [128, L], f32, kind="ExternalInput")
    biasr_d = nc.dram_tensor("biasr", [128, 8], f32, kind="ExternalInput")
    mask_d = nc.dram_tensor("maskneg", [128, 896], bf16, kind="ExternalInput")
    ones_d = nc.dram_tensor("ones", [128, 1], f16, kind="ExternalInput")
    onesb_d = nc.dram_tensor("ones_bf", [128, 1], bf16, kind="ExternalInput")
    out_d = nc.dram_tensor("out_p", [L, D], f32, kind="ExternalOutput")

    inv_sqrt_hd = float(1.0 / np.sqrt(HD))
    # exp(score - EXP_BIAS): keeps e strictly inside f16 range for any input
    # (|score| <= sqrt(HD) = 11.32 after QKNorm, exp(11.32-1.5) = 1.8e4);
    # the constant cancels exactly in the rowsum normalization.
    EXP_BIAS = -1.5

    # register float consts used as activation biases
    for cval in (EPS, 0.0, EXP_BIAS):
        if (f32, cval) in nc.const_aps.aps:
            continue
        _t = nc.alloc_sbuf_tensor(f"const-float32-{cval}", [128, 1], f32)
        nc.gpsimd.memset(_t.ap(), cval)
        nc.const_aps.aps[(f32, cval)] = _t.ap()
    nc.all_engine_barrier()

    from contextlib import ExitStack

    with ExitStack() as ctx:
        tc = ctx.enter_context(tile.TileContext(nc))
        ctx.enter_context(
            nc.allow_low_precision(
                reason="f32r rounding of matmul operands is intentional"
            )
        )
        px = ctx.enter_context(tc.tile_pool(name="px", bufs=10))     # x stream + prefetch
        pw = ctx.enter_context(tc.tile_pool(name="pw", bufs=1))       # wqk + wv
        pqk = ctx.enter_context(tc.tile_pool(name="pqk", bufs=4))     # qT/kT
        pv = ctx.enter_context(tc.tile_pool(name="pv", bufs=1))       # v slab
        pat = ctx.enter_context(tc.tile_pool(name="pat", bufs=4))     # attn_outT
        ptab = ctx.enter_context(tc.tile_pool(name="ptab", bufs=1))   # constants
        pscr = ctx.enter_context(tc.tile_pool(name="pscr", bufs=1))   # scratch
        pexp = ctx.enter_context(tc.tile_pool(name="pexp", bufs=2))   # exp tiles
        pbc = ctx.enter_context(tc.tile_pool(name="pbc", bufs=2))     # broadcasts
        psmall = ctx.enter_context(tc.tile_pool(name="psmall", bufs=2))
        ppsum = ctx.enter_context(tc.tile_pool(name="psum", bufs=3, space="PSUM"))
        ppsum_sm = ctx.enter_context(tc.tile_pool(name="psum_sm", bufs=2, space="PSUM"))
        if True:
            # ---- constant tiles (DMAs issued later, in startup-aware order)
            qcos = ptab.tile([128, L], f32, tag="qcos")
            qsin = ptab.tile([128, L], f32, tag="qsin")
            if share_cos:
                kcos, ksin = qcos, qsin
            else:
                kcos = ptab.tile([128, L], f32, tag="kcos")
                ksin = ptab.tile([128, L], f32, tag="ksin")
            biasr = ptab.tile([128, 8], f32, tag="biasr")
            masks = ptab.tile([128, 896], bf16, tag="masks")
            ones = ptab.tile([128, 1], f16, tag="ones")
            onesb = ptab.tile([128, 1], bf16, tag="onesb")

            def dma_small_consts():
                nc.sync.dma_start(masks[:], mask_d[:])
                nc.sync.dma_start(biasr[:], biasr_d[:])
                nc.sync.dma_start(ones[:], ones_d[:])
                nc.sync.dma_start(onesb[:], onesb_d[:])

            def dma_cos_consts():
                nc.sync.dma_start(qcos[:], qcos_d[:])
                nc.sync.dma_start(qsin[:], qsin_d[:])
                if not share_cos:
                    nc.sync.dma_start(kcos[:], kcos_d[:])
                    nc.sync.dma_start(ksin[:], ksin_d[:])

            attnT = {}  # (pair, hh) -> [128, L] f32r

            def gen_attn(pair, hh, j, qk, vt, aT):
                """Generator emitting one (head, q-chunk) attention unit,
                yielding after each 2-block group so projection work can be
                interleaved between groups."""
                qT = qk[(0, hh)]
                kT = qk[(1, hh)]
                jsl = slice(j * 512, j * 512 + 512)
                jhi = slice(j * 512 + 256, j * 512 + 512)
                ncc = 4 * (j + 1)
                nfull = ncc - 2     # last two diagonal chunks run half-width
                ps_o_t = ppsum.tile([128, 1024], f32, tag="big", name="ps_o")
                ps_o = ps_o_t[:, 0:512]
                ps_sum = ps_o_t[0:1, 512:1024]
                # software-pipelined: scores+exp for group g are emitted one
                # iteration ahead of g's av/rowsum, so the PE runs group
                # g+1's scores while the scalar engine exps group g
                pending_av = None   # (e_tile, cp) awaiting av/rowsum

                def emit_av(e, cp, width, qoff):
                    for ci in range(2):
                        c = cp + ci
                        esl = e[:, ci * width : ci * width + width]
                        nc.tensor.matmul(
                            ps_sum[:, qoff : qoff + width], ones[:], esl,
                            start=(c == 0), stop=(c == ncc - 1),
                            skip_group_check=True,
                        )
                        nc.tensor.matmul(
                            ps_o[:, qoff : qoff + width], vt[hh][:, c, :], esl,
                            start=(c == 0), stop=(c == ncc - 1),
                            skip_group_check=True,
                        )

                for cp in range(0, nfull, 2):
                    ps_s = ppsum.tile([128, 1024], f32, tag="big", name="ps_s")
                    for ci in range(2):
                        c = cp + ci
                        nc.tensor.matmul(
                            ps_s[:, ci * 512 : ci * 512 + 512],
                            kT[:, c * 128 : c * 128 + 128],
                            qT[:, jsl],
                            start=True,
                            stop=True,
                        )
                        r = c - 4 * j
                        if r >= 0:
                            ms0 = 384 - 128 * r
                            nc.vector.tensor_tensor(
                                ps_s[:, ci * 512 : ci * 512 + 512],
                                ps_s[:, ci * 512 : ci * 512 + 512],
                                masks[:, ms0 : ms0 + 512],
                                ALU.add,
                            )
                    e = pexp.tile([128, 1024], f16, tag="e")
                    for ci in range(2):
                        nc.scalar.activation(
                            e[:, ci * 512 : ci * 512 + 512],
                            ps_s[:, ci * 512 : ci * 512 + 512],
                            AF.Exp, scale=inv_sqrt_hd, bias=EXP_BIAS,
                        )
                    if pending_av is not None:
                        emit_av(*pending_av)
                    pending_av = (e, cp, 512, 0)
                    yield
                # diagonal pair (r=2,3): q columns [0,256) of the upper half
                # are fully masked, so only compute the top 256 q columns
                ps_s = ppsum.tile([128, 1024], f32, tag="big", name="ps_s")
                for ci in range(2):
                    c = nfull + ci
                    nc.tensor.matmul(
                        ps_s[:, ci * 256 : ci * 256 + 256],
                        kT[:, c * 128 : c * 128 + 128],
                        qT[:, jhi],
                        start=True,
                        stop=True,
                    )
                    # valid iff q >= k: within this [128, 256] tile the
                    # allowed region is u' >= i + 128*(ci-1) + 128
                    ms0 = 384 - 128 * ci
                    nc.vector.tensor_tensor(
                        ps_s[:, ci * 256 : ci * 256 + 256],
                        ps_s[:, ci * 256 : ci * 256 + 256],
                        masks[:, ms0 : ms0 + 256],
                        ALU.add,
                    )
                e = pexp.tile([128, 1024], f16, tag="e")
                nc.scalar.activation(
                    e[:, 0:512], ps_s[:, 0:512], AF.Exp, scale=inv_sqrt_hd,
                    bias=EXP_BIAS,
                )
                if pending_av is not None:
                    emit_av(*pending_av)
                emit_av(e, nfull, 256, 256)
                yield
                # normalize straight out of PSUM (fused copy+scale)
                reca = psmall.tile([1, 512], f32, tag="small")
                nc.vector.reciprocal_approx_fast(reca[:], ps_sum)
                rbc = pbc.tile([128, 512], f32, tag="bc")
                nc.gpsimd.partition_broadcast(rbc[:], reca[:])
                nc.vector.tensor_tensor(aT[:, jsl], ps_o, rbc[:], ALU.mult)

            from collections import deque

            gens = deque()
            pend = [None]   # deferred norm stageB, carried across c4/pair

            def flush_pend():
                if pend[0] is not None:
                    pend[0]()
                    pend[0] = None

            def advance(n):
                for _ in range(n):
                    while gens:
                        try:
                            next(gens[0][1])
                            break
                        except StopIteration:
                            gens.popleft()
                    if not gens:
                        break

            def drain():
                while gens:
                    try:
                        next(gens[0][1])
                    except StopIteration:
                        gens.popleft()

            def drain_until(j_max):
                # fully drain every pending generator labelled <= j_max
                # (FIFO front is always oldest; labels are per-pair ascending)
                while any(lbl <= j_max for lbl, _ in gens):
                    try:
                        next(gens[0][1])
                    except StopIteration:
                        gens.popleft()

            xpre = []   # [(tile, kh)] prefetched for the next c4

            def prefetch_x(pair_, c4_):
                t0_ = c4_ * 512
                for kh in range(2):
                    xt = px.tile([128, 2, 512], f32r, tag="x", name=f"xp_{kh}")
                    nc.sync.dma_start(
                        xt[:], xT_d[:, 2 * kh : 2 * kh + 2, t0_ : t0_ + 512]
                    )
                    xpre.append(xt)

            for pair in range(2):
                # ---- weights for this pair (wv first: v-proj runs first) ----
                wqk = pw.tile([128, KC, 512], f32r, tag="wqk")
                wv = pw.tile([128, KC, 256], f32r, tag="wv")
                nc.sync.dma_start(wv[:, 0 : KC // 4], wv_d[pair][:, 0 : KC // 4])
                if pair != 0:
                    for kq in range(1, 4):
                        ksl = slice(kq * (KC // 4), (kq + 1) * (KC // 4))
                        nc.sync.dma_start(wv[:, ksl], wv_d[pair][:, ksl])
                    for kq in range(4):
                        ksl = slice(kq * (KC // 4), (kq + 1) * (KC // 4))
                        nc.sync.dma_start(wqk[:, ksl], wqk_d[pair][:, ksl])

                if pair != 0:
                    # pair 0's pending attention reads the v/qk slabs this
                    # pair is about to overwrite: flush the deferred norm
                    # write first, then drain the OLDEST units so emission
                    # order stays write-after-read safe; the rest drains
                    # during this pair's projections
                    flush_pend()
                    drain_until(3)
                vt = [
                    pv.tile([128, TK, 128], f16, tag=f"v{hh}", name=f"v_{hh}")
                    for hh in range(2)
                ]
                qk = {}
                for qki in range(2):
                    for hh in range(2):
                        qk[(qki, hh)] = pqk.tile(
                            [128, L], f32r, tag="qk", name=f"qk_{qki}_{hh}"
                        )
                aTs = {}
                for hh in range(2):
                    aTs[hh] = pat.tile([128, L], f32r, tag="attnT", name=f"aT_{hh}")
                    attnT[(pair, hh)] = aTs[hh]

                # ---- projection + norm + rope + interleaved attention ----
                for c4 in range(TQ):
                    t0 = c4 * 512
                    if xpre:
                        xts = list(xpre)
                        xpre.clear()
                    else:
                        xts = []
                        for kh in range(2):
                            xt = px.tile(
                                [128, 2, 512], f32r, tag="x", name=f"xt_{kh}"
                            )
                            nc.sync.dma_start(
                                xt[:], xT_d[:, 2 * kh : 2 * kh + 2, t0 : t0 + 512]
                            )
                            xts.append(xt)
                    for kh in range(2, 8):
                        xt = px.tile([128, 2, 512], f32r, tag="x", name=f"xt_{kh}")
                        nc.sync.dma_start(
                            xt[:], xT_d[:, 2 * kh : 2 * kh + 2, t0 : t0 + 512]
                        )
                        xts.append(xt)
                    if pair == 0 and c4 == 0:
                        # startup: x + wv chunk 0 already queued; rest follows
                        for kq in range(1, 4):
                            ksl = slice(kq * (KC // 4), (kq + 1) * (KC // 4))
                            nc.sync.dma_start(wv[:, ksl], wv_d[pair][:, ksl])
                        dma_small_consts()
                        for kq in range(4):
                            ksl = slice(kq * (KC // 4), (kq + 1) * (KC // 4))
                            nc.sync.dma_start(wqk[:, ksl], wqk_d[pair][:, ksl])
                        dma_cos_consts()

                    def norm_stageA(ps, bidx):
                        # biased projection staged to SBUF; the single DVE op
                        # is the only PSUM reader, so the bank frees fast and
                        # the next projection group never waits on ACT
                        q_sb = pscr.tile([128, 512], f32, tag="q_sb")
                        nc.vector.tensor_scalar_add(
                            q_sb[:], ps[:], biasr[:, bidx : bidx + 1]
                        )
                        # squares for the rms-norm sum, also on DVE so the ms
                        # matmul never waits behind ACT's exp queue
                        sq = pscr.tile([128, 512], bf16, tag="sq")
                        nc.vector.tensor_tensor(sq[:], q_sb[:], q_sb[:], ALU.mult)
                        rot = pscr.tile([128, 512], f32, tag="rot")
                        nc.sync.dma_start(rot[0:64, :], q_sb[64:128, :])
                        nc.sync.dma_start(rot[64:128, :], q_sb[0:64, :])
                        return q_sb, sq, rot

                    def norm_stageB(q_sb, sq, rot, qki, dst, tsl):
                        cosT = qcos if qki == 0 else kcos
                        sinT = qsin if qki == 0 else ksin
                        t1 = pscr.tile([128, 512], f32, tag="t1")
                        nc.vector.tensor_tensor(t1[:], q_sb[:], cosT[:, tsl], ALU.mult)
                        nc.vector.tensor_tensor(rot[:], rot[:], sinT[:, tsl], ALU.mult)
                        # deferred: the ms matmul runs once ACT's square is
                        # long done, so PE never stalls on it
                        ms = ppsum_sm.tile([1, 512], f32, tag="sm", name="ms")
                        nc.tensor.matmul(
                            ms[:], onesb[:], sq[:], start=True, stop=True
                        )
                        # rstd = exp(-0.5*ln(ms/HD + eps)): Ln and Exp share
                        # one activation-function set with Square/Copy, so
                        # the scalar engine never reloads tables (Sqrt would)
                        lms = psmall.tile([1, 512], f32, tag="small")
                        nc.scalar.activation(
                            lms[:], ms[:], AF.Ln, bias=EPS, scale=float(1.0 / HD)
                        )
                        rec = psmall.tile([1, 512], f32, tag="small")
                        nc.scalar.activation(rec[:], lms[:], AF.Exp, scale=-0.5)
                        rstd = pbc.tile([128, 512], f32, tag="bc")
                        nc.gpsimd.partition_broadcast(rstd[:], rec[:])
                        nc.vector.tensor_tensor(t1[:], t1[:], rot[:], ALU.add)
                        nc.vector.tensor_tensor(
                            dst[:, tsl], t1[:], rstd[:], ALU.mult
                        )

                    # v projection first (attention j=c4 needs v chunks <= c4)
                    for tsub in range(4):
                        ps_v = ppsum_sm.tile([128, 256], f32, tag="sm", name="ps_v")
                        for kc in range(KC):
                            nc.tensor.matmul(
                                ps_v[:],
                                xts[kc // 2][:, kc % 2, tsub * 128 : tsub * 128 + 128],
                                wv[:, kc, :],
                                start=(kc == 0),
                                stop=(kc == KC - 1),
                            )
                        vi = c4 * 4 + tsub
                        for hh in range(2):
                            nc.vector.tensor_copy(
                                vt[hh][:, vi, :],
                                ps_v[:, hh * 128 : hh * 128 + 128],
                            )
                        if tsub == 0:
                            flush_pend()
                        advance(4)

                    # prefetch the next c4's first x tiles while the qk
                    # phase runs (the v phase is x-DMA-paced otherwise)
                    if c4 < TQ - 1:
                        prefetch_x(pair, c4 + 1)
                    elif pair == 0:
                        prefetch_x(1, 0)

                    # q/k projections + split norm/rope, one instance at a time
                    for qki in range(2):
                        for hh in range(2):
                            ps = ppsum_sm.tile(
                                [128, 512], f32, tag="sm", name=f"psqk_{qki}_{hh}"
                            )
                            col = (qki * 2 + hh) * 128
                            for kc in range(KC):
                                nc.tensor.matmul(
                                    ps[:],
                                    wqk[:, kc, col : col + 128],
                                    xts[kc // 2][:, kc % 2, :],
                                    start=(kc == 0),
                                    stop=(kc == KC - 1),
                                )
                            bidx = pair * 4 + qki * 2 + hh
                            flush_pend()
                            tsl = slice(c4 * 512, c4 * 512 + 512)
                            q_sb, sq, rot = norm_stageA(ps, bidx)
                            pend[0] = (
                                lambda q_sb=q_sb, sq=sq, rot=rot, qki=qki,
                                dst=qk[(qki, hh)], tsl=tsl: norm_stageB(
                                    q_sb, sq, rot, qki, dst, tsl
                                )
                            )
                            advance(3)

                    for hh in range(2):
                        gens.append((c4, gen_attn(pair, hh, c4, qk, vt, aTs[hh])))
            flush_pend()
            # ---- out projection, interleaved with the attention drain ----
            # stream w_out into the freed wv (heads 0,1) / wqk (heads 2,3)
            # slots, chunked along D so the first matmuls start early
            wo_a = pw.tile([128, 2, D], f32r, tag="wv", name="wo_a")
            wo_b = pw.tile([128, 2, D], f32r, tag="wqk", name="wo_b")
            for n4 in range(4):
                nsl = slice(n4 * 512, n4 * 512 + 512)
                nc.sync.dma_start(wo_a[:, :, nsl], wout_d[:, 0:2, nsl])
                nc.sync.dma_start(wo_b[:, :, nsl], wout_d[:, 2:4, nsl])
            wo = {0: wo_a, 1: wo_a, 2: wo_b, 3: wo_b}
            for tc_i in range(TK):
                # attnT writes for this t-chunk must be EMITTED before the
                # reads (Tile deps follow emission order)
                drain_until(tc_i // 4)
                tsl = slice(tc_i * 128, tc_i * 128 + 128)
                for n2 in range(D // 1024):
                    ps = ppsum.tile([128, 1024], f32, tag="big", name="ps_out")
                    for half in range(2):
                        nsl = slice(n2 * 1024 + half * 512, n2 * 1024 + half * 512 + 512)
                        for hi in range(4):
                            nc.tensor.matmul(
                                ps[:, half * 512 : half * 512 + 512],
                                attnT[(hi // 2, hi % 2)][:, tsl],
                                wo[hi][:, hi % 2, nsl],
                                start=(hi == 0),
                                stop=(hi == 3),
                            )
                    # past tc 12 the generators are fully drained, so the
                    # idle exp-tile slots can double the staging depth (the
                    # out DMA is the binding latency in the tail)
                    if tc_i >= 12 and (tc_i * 2 + n2) % 2 == 0:
                        o = pexp.tile([128, 1024], f32, tag="e", name="o_stage")
                    else:
                        o = px.tile([128, 1024], f32, tag="x", name="o_stage")
                    if (tc_i * 2 + n2) % 2 == 0:
                        nc.vector.tensor_copy(o[:], ps[:])
                    else:
                        nc.scalar.copy(o[:], ps[:])
                    nc.sync.dma_start(
                        out_d[tsl, n2 * 1024 : n2 * 1024 + 1024], o[:]
                    )
                    advance(2)
            drain()

    nc.compile()
    return nc


def host_inputs(x, w_qkv, b_qkv, q_scale, k_scale, w_out, L, share_cos=True):
    """Build the 8 per-core input maps."""
    x = np.asarray(x, np.float32)
    w_qkv = np.asarray(w_qkv, np.float32)
    b_qkv = np.asarray(b_qkv, np.float32)
    w_out = np.asarray(w_out, np.float32)
    q_scale = np.asarray(q_scale, np.float32)
    k_scale = np.asarray(k_scale, np.float32)

    half = HD // 2
    inv_freq = 1.0 / (ROPE_THETA ** (np.arange(half, dtype=np.float64) / half))
    pos = np.arange(L, dtype=np.float64)
    ang = pos[None, :] * inv_freq[:, None]          # [64, L]
    cos_t = np.cos(ang)
    sin_t = np.sin(ang)
    import ml_dtypes
    cosT = np.concatenate([cos_t, cos_t], 0)            # [128, L]
    sinT = np.concatenate([-sin_t, sin_t], 0)           # [-s; +s]
    # fold the qk-norm output scales into the rope tables: output dim i is
    # (pre-rope dim i)*cos_i + (pre-rope dim (i+64)%128)*sin_i, all scaled
    # by scale_i.
    def fold(scale):
        c = (cosT * scale[:, None]).astype(np.float32)
        s = (sinT * scale[:, None]).astype(np.float32)
        return np.ascontiguousarray(c), np.ascontiguousarray(s)

    qcosT, qsinT = fold(q_scale)
    kcosT, ksinT = fold(k_scale)

    # consolidated straddle mask: M[i, u] = 0 iff u >= i + 384 else NEG;
    # slice [384-128r : 896-128r] gives the r-straddle [128, 512] mask
    ii = np.arange(128)[:, None]
    uu = np.arange(896)[None, :]
    maskneg = np.ascontiguousarray(
        np.where(uu >= ii + 384, 0.0, NEG).astype(ml_dtypes.bfloat16)
    )

    onesv = np.ones((128, 1), np.float16)
    onesb = np.ones((128, 1), ml_dtypes.bfloat16)

    in_maps = []
    for c in range(N_CORES):
        b = c // 4
        g = c % 4
        heads = [4 * g + i for i in range(4)]
        xT = np.ascontiguousarray(x[b].T)                       # [D, L]
        xTr = np.ascontiguousarray(
            xT.reshape(KC, 128, L).transpose(1, 0, 2)
        )                                                        # [128, KC, L]
        wqk = np.empty((2, 128, KC, 512), np.float32)
        wv = np.empty((2, 128, KC, 256), np.float32)
        biasr = np.empty((128, 8), np.float32)   # [hd, (pair, qki, hh)]
        for p in range(2):
            hp = heads[2 * p : 2 * p + 2]
            cols = np.concatenate(
                [
                    np.arange(qki * D + h * HD, qki * D + (h + 1) * HD)
                    for qki in range(2)
                    for h in hp
                ]
            )
            wqk[p] = w_qkv[:, cols].reshape(KC, 128, 512).transpose(1, 0, 2)
            vcols = np.concatenate(
                [np.arange(2 * D + h * HD, 2 * D + (h + 1) * HD) for h in hp]
            )
            wv[p] = w_qkv[:, vcols].reshape(KC, 128, 256).transpose(1, 0, 2)
            for qki in range(2):
                for hh in range(2):
                    biasr[:, p * 4 + qki * 2 + hh] = b_qkv[
                        qki * D + hp[hh] * HD : qki * D + (hp[hh] + 1) * HD
                    ]

        wout = (
            w_out[heads[0] * HD : (heads[-1] + 1) * HD]
            .reshape(4, 128, D)
            .transpose(1, 0, 2)
        )
        in_maps.append(
            {
                "xT": np.ascontiguousarray(xTr),
                "wqk": np.ascontiguousarray(wqk),
                "wv": np.ascontiguousarray(wv),
                "wout": np.ascontiguousarray(wout),
                "qcosT": qcosT,
                "qsinT": qsinT,
                **({} if share_cos else {"kcosT": kcosT, "ksinT": ksinT}),
                "biasr": np.ascontiguousarray(biasr),
                "maskneg": maskneg,
                "ones": onesv,
                "ones_bf": onesb,
            }
        )
    return in_maps


_NC_CACHE = {}


def _get_nc(L, share_cos=True):
    key = (L, share_cos)
    if key not in _NC_CACHE:
        _NC_CACHE[key] = build_nc(L, share_cos=share_cos)
    return _NC_CACHE[key]


def run(x, w_qkv, b_qkv, q_scale, k_scale, w_out, b_out, L, **rb_kwargs):
    share_cos = bool(np.array_equal(np.asarray(q_scale), np.asarray(k_scale)))
    nc = _get_nc(L, share_cos)
    in_maps = host_inputs(x, w_qkv, b_qkv, q_scale, k_scale, w_out, L,
                          share_cos=share_cos)
    res = run_bass_kernel_spmd(nc, in_maps, list(range(N_CORES)), **rb_kwargs)
    parts = np.stack([r["out_p"] for r in res.results])          # [8, L, D]
    b_v = np.asarray(b_qkv, np.float64)[2 * D : 3 * D]
    bias_eff = np.asarray(b_out, np.float64) + b_v @ np.asarray(w_out, np.float64)
    out = np.empty((B, L, D), np.float32)
    for b in range(B):
        out[b] = parts[4 * b : 4 * b + 4].sum(0, dtype=np.float64) + bias_eff
    return out, res


def kernel(x, w_qkv, b_qkv, q_scale, k_scale, w_out, b_out, mask):
    out, _ = run(x, w_qkv, b_qkv, q_scale, k_scale, w_out, b_out, L=x.shape[1])
    return out



# revision 18
# speedup vs baseline: 1.0339x; 1.0339x over previous
"""Trainium2 Bass kernel for fused attention layer (QKV proj + QKNorm + RoPE +
causal attention + out proj), tensor-parallel across (batch, head-group) on 8
NeuronCores.

Reference semantics (B=2, L=2048, D=2048, H=16, HD=128):
    qkv = x @ w_qkv + b_qkv ; split q,k,v per head
    q,k = rms_norm(q)*q_scale, rms_norm(k)*k_scale   (over head_dim)
    q,k = rope(q), rope(k)                           (rotate-half)
    attn = softmax(mask(q k^T / sqrt(HD)))
    out = (attn @ v) reshaped @ w_out + b_out
Sharding: core c -> batch b=c//4, heads 4g..4g+3 with g=c%4. Each core emits a
partial out-projection [L, D]; the host sums the 4 partials per batch + b_out.

v2 scheduling notes:
  - qkv bias is applied by a 17th 1-partition matmul in each accumulation
    group (stationary = bias row, moving = a [1,512] ones vector), so the
    norm/rope pipeline reads the biased projection directly from PSUM.
  - q_scale/k_scale are folded into per-q/k cos/sin tables on the host
    (rope output dim i scales by s_i, which multiplies cos_i and the sin
    factor sourced from partition (i+64)%128).
  - DMA issue order puts the first-consumed bytes (wv chunk 0, x tiles)
    ahead of wqk/constants so the PE starts ~3us in.
  - w_out streams into the freed wv/wqk slots right after the last
    projection reads, chunked so the first out-proj matmuls start early.
  - out projection DMAs straight from PSUM (no staging copy).
"""

import numpy as np

import concourse.bass as bass
import concourse.bacc as bacc
import concourse.tile as tile
import concourse.mybir as mybir
from concourse.bass_utils import run_bass_kernel_spmd

# This kernel only uses Square/Ln/Exp/Copy/Identity activations, all present
# in the 'natural_log_exp_and_others' set. The greedy per-function set
# chooser would otherwise ping-pong between 'exp_and_others' and
# 'natural_log' (a ~1.3us table load per switch, on the softmax critical
# path). Restrict candidacy to the one set that covers everything; entry
# order (= act_func_set_id) is preserved so walrus still loads the right
# table.
_orig_get_act_tables = bacc.get_activation_tables


def _act_tables_one_set(arch):
    tables = _orig_get_act_tables(arch)
    keep = "natural_log_exp_and_others"
    if keep in tables:
        tables = {
            name: (funcs if name == keep else type(funcs)())
            for name, funcs in tables.items()
        }
    return tables


bacc.get_activation_tables = _act_tables_one_set

f32 = mybir.dt.float32
f16 = mybir.dt.float16
f32r = mybir.dt.float32r
bf16 = mybir.dt.bfloat16
AF = mybir.ActivationFunctionType
ALU = mybir.AluOpType

B = 2
D = 2048
H = 16
HD = 128
KC = D // 128          # 16 contraction chunks of 128
EPS = 1e-6
ROPE_THETA = 10000.0
NEG = -1e30
N_CORES = 8
HEADS_PER_CORE = 4     # 2 pairs of 2


def build_nc(L, share_cos=True):
    TQ = L // 512       # 512-wide t chunks (query chunks, proj chunks)
    TK = L // 128       # 128-wide t chunks (key chunks, v chunks, out-proj chunks)

    nc = bacc.Bacc(None, target_bir_lowering=False)

    xT_d = nc.dram_tensor("xT", [128, KC, L], f32r, kind="ExternalInput")
    wqk_d = nc.dram_tensor("wqk", [2, 128, KC, 512], f32r, kind="ExternalInput")
    wv_d = nc.dram_tensor("wv", [2, 128, KC, 256], f32r, kind="ExternalInput")
    wout_d = nc.dram_tensor("wout", [128, 4, D], f32r, kind="ExternalInput")
    qcos_d = nc.dram_tensor("qcosT", [128, L], f32, kind="ExternalInput")
    qsin_d = nc.dram_tensor("qsinT", [128, L], f32, kind="ExternalInput")
    if not share_cos:
        kcos_d = nc.dram_tensor("kcosT", [128, L], f32, kind="ExternalInput")
        ksin_d = nc.dram_tensor("ksinT", [128, L], f32, kind="ExternalInput")
    biasr_d = nc.dram_tensor("biasr", [128, 8], f32, kind="ExternalInput")
    mask_d = nc.dram_tensor("maskneg", [128, 896], bf16, kind="ExternalInput")
    ones_d = nc.dram_tensor("ones", [128, 1], f32r, kind="ExternalInput")
    onesb_d = nc.dram_tensor("ones_bf", [128, 1], bf16, kind="ExternalInput")
    out_d = nc.dram_tensor("out_p", [L, D], f32, kind="ExternalOutput")

    inv_sqrt_hd = float(1.0 / np.sqrt(HD))
    # exp(score - EXP_BIAS): keeps e strictly inside f16 range for any input
    # (|score| <= sqrt(HD) = 11.32 after QKNorm, exp(11.32-1.5) = 1.8e4);
    # the constant cancels exactly in the rowsum normalization.
    EXP_BIAS = -1.5

    # register float consts used as activation biases
    for cval in (EPS, 0.0, EXP_BIAS):
        if (f32, cval) in nc.const_aps.aps:
            continue
        _t = nc.alloc_sbuf_tensor(f"const-float32-{cval}", [128, 1], f32)
        nc.gpsimd.memset(_t.ap(), cval)
        nc.const_aps.aps[(f32, cval)] = _t.ap()
    nc.all_engine_barrier()

    from contextlib import ExitStack

    with ExitStack() as ctx:
        tc = ctx.enter_context(tile.TileContext(nc))
        ctx.enter_context(
            nc.allow_low_precision(
                reason="f32r rounding of matmul operands is intentional"
            )
        )
        px = ctx.enter_context(tc.tile_pool(name="px", bufs=8))      # x stream
        pw = ctx.enter_context(tc.tile_pool(name="pw", bufs=1))       # wqk + wv
        pqk = ctx.enter_context(tc.tile_pool(name="pqk", bufs=4))     # qT/kT
        pv = ctx.enter_context(tc.tile_pool(name="pv", bufs=1))       # v slab
        pat = ctx.enter_context(tc.tile_pool(name="pat", bufs=4))     # attn_outT
        ptab = ctx.enter_context(tc.tile_pool(name="ptab", bufs=1))   # constants
        pscr = ctx.enter_context(tc.tile_pool(name="pscr", bufs=1))   # scratch
        pexp = ctx.enter_context(tc.tile_pool(name="pexp", bufs=2))   # exp tiles
        pbc = ctx.enter_context(tc.tile_pool(name="pbc", bufs=2))     # broadcasts
        psmall = ctx.enter_context(tc.tile_pool(name="psmall", bufs=2))
        ppsum = ctx.enter_context(tc.tile_pool(name="psum", bufs=3, space="PSUM"))
        ppsum_sm = ctx.enter_context(tc.tile_pool(name="psum_sm", bufs=2, space="PSUM"))
        if True:
            # ---- constant tiles (DMAs issued later, in startup-aware order)
            qcos = ptab.tile([128, L], f32, tag="qcos")
            qsin = ptab.tile([128, L], f32, tag="qsin")
            if share_cos:
                kcos, ksin = qcos, qsin
            else:
                kcos = ptab.tile([128, L], f32, tag="kcos")
                ksin = ptab.tile([128, L], f32, tag="ksin")
            biasr = ptab.tile([128, 8], f32, tag="biasr")
            masks = ptab.tile([128, 896], bf16, tag="masks")
            ones = ptab.tile([128, 1], f32r, tag="ones")
            onesb = ptab.tile([128, 1], bf16, tag="onesb")

            def dma_small_consts():
                nc.sync.dma_start(masks[:], mask_d[:])
                nc.sync.dma_start(biasr[:], biasr_d[:])
                nc.sync.dma_start(ones[:], ones_d[:])
                nc.sync.dma_start(onesb[:], onesb_d[:])

            def dma_cos_consts():
                nc.sync.dma_start(qcos[:], qcos_d[:])
                nc.sync.dma_start(qsin[:], qsin_d[:])
                if not share_cos:
                    nc.sync.dma_start(kcos[:], kcos_d[:])
                    nc.sync.dma_start(ksin[:], ksin_d[:])

            attnT = {}  # (pair, hh) -> [128, L] f32r

            def gen_attn(pair, hh, j, qk, vt, aT):
                """Generator emitting one (head, q-chunk) attention unit,
                yielding after each 2-block group so projection work can be
                interleaved between groups."""
                qT = qk[(0, hh)]
                kT = qk[(1, hh)]
                jsl = slice(j * 512, j * 512 + 512)
                jhi = slice(j * 512 + 256, j * 512 + 512)
                ncc = 4 * (j + 1)
                nfull = ncc - 2     # last two diagonal chunks run half-width
                ps_o_t = ppsum.tile([128, 1024], f32, tag="big", name="ps_o")
                ps_o = ps_o_t[:, 0:512]
                ps_sum = ps_o_t[0:1, 512:1024]
                # software-pipelined: scores+exp for group g are emitted one
                # iteration ahead of g's av/rowsum, so the PE runs group
                # g+1's scores while the scalar engine exps group g
                pending_av = None   # (e_tile, cp) awaiting av/rowsum

                def emit_av(e, cp, width, qoff):
                    for ci in range(2):
                        c = cp + ci
                        esl = e[:, ci * width : ci * width + width]
                        nc.tensor.matmul(
                            ps_sum[:, qoff : qoff + width], ones[:], esl,
                            start=(c == 0), stop=(c == ncc - 1),
                            skip_group_check=True,
                        )
                        nc.tensor.matmul(
                            ps_o[:, qoff : qoff + width], vt[hh][:, c, :], esl,
                            start=(c == 0), stop=(c == ncc - 1),
                            skip_group_check=True,
                        )

                for cp in range(0, nfull, 2):
                    ps_s = ppsum.tile([128, 1024], f32, tag="big", name="ps_s")
                    for ci in range(2):
                        c = cp + ci
                        nc.tensor.matmul(
                            ps_s[:, ci * 512 : ci * 512 + 512],
                            kT[:, c * 128 : c * 128 + 128],
                            qT[:, jsl],
                            start=True,
                            stop=True,
                        )
                        r = c - 4 * j
                        if r >= 0:
                            ms0 = 384 - 128 * r
                            nc.vector.tensor_tensor(
                                ps_s[:, ci * 512 : ci * 512 + 512],
                                ps_s[:, ci * 512 : ci * 512 + 512],
                                masks[:, ms0 : ms0 + 512],
                                ALU.add,
                            )
                    e = pexp.tile([128, 1024], f32r, tag="e")
                    for ci in range(2):
                        nc.scalar.activation(
                            e[:, ci * 512 : ci * 512 + 512],
                            ps_s[:, ci * 512 : ci * 512 + 512],
                            AF.Exp, scale=inv_sqrt_hd, bias=EXP_BIAS,
                        )
                    if pending_av is not None:
                        emit_av(*pending_av)
                    pending_av = (e, cp, 512, 0)
                    yield
                # diagonal pair (r=2,3): q columns [0,256) of the upper half
                # are fully masked, so only compute the top 256 q columns
                ps_s = ppsum.tile([128, 1024], f32, tag="big", name="ps_s")
                for ci in range(2):
                    c = nfull + ci
                    nc.tensor.matmul(
                        ps_s[:, ci * 256 : ci * 256 + 256],
                        kT[:, c * 128 : c * 128 + 128],
                        qT[:, jhi],
                        start=True,
                        stop=True,
                    )
                    # valid iff q >= k: within this [128, 256] tile the
                    # allowed region is u' >= i + 128*(ci-1) + 128
                    ms0 = 384 - 128 * ci
                    nc.vector.tensor_tensor(
                        ps_s[:, ci * 256 : ci * 256 + 256],
                        ps_s[:, ci * 256 : ci * 256 + 256],
                        masks[:, ms0 : ms0 + 256],
                        ALU.add,
                    )
                e = pexp.tile([128, 1024], f32r, tag="e")
                nc.scalar.activation(
                    e[:, 0:512], ps_s[:, 0:512], AF.Exp, scale=inv_sqrt_hd,
                    bias=EXP_BIAS,
                )
                if pending_av is not None:
                    emit_av(*pending_av)
                emit_av(e, nfull, 256, 256)
                yield
                # normalize straight out of PSUM (fused copy+scale)
                reca = psmall.tile([1, 512], f32, tag="small")
                nc.vector.reciprocal_approx_fast(reca[:], ps_sum)
                rbc = pbc.tile([128, 512], f32, tag="bc")
                nc.gpsimd.partition_broadcast(rbc[:], reca[:])
                nc.vector.tensor_tensor(aT[:, jsl], ps_o, rbc[:], ALU.mult)

            from collections import deque

            gens = deque()
            pend = [None]   # deferred norm stageB, carried across c4/pair

            def flush_pend():
                if pend[0] is not None:
                    pend[0]()
                    pend[0] = None

            def advance(n):
                for _ in range(n):
                    while gens:
                        try:
                            next(gens[0][1])
                            break
                        except StopIteration:
                            gens.popleft()
                    if not gens:
                        break

            def drain():
                while gens:
                    try:
                        next(gens[0][1])
                    except StopIteration:
                        gens.popleft()

            def drain_until(j_max):
                # fully drain every pending generator labelled <= j_max
                # (FIFO front is always oldest; labels are per-pair ascending)
                while any(lbl <= j_max for lbl, _ in gens):
                    try:
                        next(gens[0][1])
                    except StopIteration:
                        gens.popleft()

            for pair in range(2):
                # ---- weights for this pair (wv first: v-proj runs first) ----
                wqk = pw.tile([128, KC, 512], f32r, tag="wqk")
                wv = pw.tile([128, KC, 256], f32r, tag="wv")
                nc.sync.dma_start(wv[:, 0 : KC // 4], wv_d[pair][:, 0 : KC // 4])
                if pair != 0:
                    for kq in range(1, 4):
                        ksl = slice(kq * (KC // 4), (kq + 1) * (KC // 4))
                        nc.sync.dma_start(wv[:, ksl], wv_d[pair][:, ksl])
                    for kq in range(4):
                        ksl = slice(kq * (KC // 4), (kq + 1) * (KC // 4))
                        nc.sync.dma_start(wqk[:, ksl], wqk_d[pair][:, ksl])

                if pair != 0:
                    # pair 0's pending attention reads the v/qk slabs this
                    # pair is about to overwrite: flush the deferred norm
                    # write first, then drain the OLDEST units so emission
                    # order stays write-after-read safe; the rest drains
                    # during this pair's projections
                    flush_pend()
                    drain_until(3)
                vt = [
                    pv.tile([128, TK, 128], f32r, tag=f"v{hh}", name=f"v_{hh}")
                    for hh in range(2)
                ]
                qk = {}
                for qki in range(2):
                    for hh in range(2):
                        qk[(qki, hh)] = pqk.tile(
                            [128, L], f32r, tag="qk", name=f"qk_{qki}_{hh}"
                        )
                aTs = {}
                for hh in range(2):
                    aTs[hh] = pat.tile([128, L], f32r, tag="attnT", name=f"aT_{hh}")
                    attnT[(pair, hh)] = aTs[hh]

                # ---- projection + norm + rope + interleaved attention ----
                for c4 in range(TQ):
                    t0 = c4 * 512
                    xts = []
                    for kh in range(8):
                        xt = px.tile([128, 2, 512], f32r, tag="x", name=f"xt_{kh}")
                        nc.sync.dma_start(
                            xt[:], xT_d[:, 2 * kh : 2 * kh + 2, t0 : t0 + 512]
                        )
                        xts.append(xt)
                    if pair == 0 and c4 == 0:
                        # startup: x + wv chunk 0 already queued; rest follows
                        for kq in range(1, 4):
                            ksl = slice(kq * (KC // 4), (kq + 1) * (KC // 4))
                            nc.sync.dma_start(wv[:, ksl], wv_d[pair][:, ksl])
                        dma_small_consts()
                        for kq in range(4):
                            ksl = slice(kq * (KC // 4), (kq + 1) * (KC // 4))
                            nc.sync.dma_start(wqk[:, ksl], wqk_d[pair][:, ksl])
                        dma_cos_consts()

                    def norm_stageA(ps, bidx):
                        # biased projection staged to SBUF; the single DVE op
                        # is the only PSUM reader, so the bank frees fast and
                        # the next projection group never waits on ACT
                        q_sb = pscr.tile([128, 512], f32, tag="q_sb")
                        nc.vector.tensor_scalar_add(
                            q_sb[:], ps[:], biasr[:, bidx : bidx + 1]
                        )
                        # squares for the rms-norm sum, also on DVE so the ms
                        # matmul never waits behind ACT's exp queue
                        sq = pscr.tile([128, 512], bf16, tag="sq")
                        nc.vector.tensor_tensor(sq[:], q_sb[:], q_sb[:], ALU.mult)
                        rot = pscr.tile([128, 512], f32, tag="rot")
                        nc.sync.dma_start(rot[0:64, :], q_sb[64:128, :])
                        nc.sync.dma_start(rot[64:128, :], q_sb[0:64, :])
                        return q_sb, sq, rot

                    def norm_stageB(q_sb, sq, rot, qki, dst, tsl):
                        cosT = qcos if qki == 0 else kcos
                        sinT = qsin if qki == 0 else ksin
                        t1 = pscr.tile([128, 512], f32, tag="t1")
                        nc.vector.tensor_tensor(t1[:], q_sb[:], cosT[:, tsl], ALU.mult)
                        nc.vector.tensor_tensor(rot[:], rot[:], sinT[:, tsl], ALU.mult)
                        # deferred: the ms matmul runs once ACT's square is
                        # long done, so PE never stalls on it
                        ms = ppsum_sm.tile([1, 512], f32, tag="sm", name="ms")
                        nc.tensor.matmul(
                            ms[:], onesb[:], sq[:], start=True, stop=True
                        )
                        # rstd = exp(-0.5*ln(ms/HD + eps)): Ln and Exp share
                        # one activation-function set with Square/Copy, so
                        # the scalar engine never reloads tables (Sqrt would)
                        lms = psmall.tile([1, 512], f32, tag="small")
                        nc.scalar.activation(
                            lms[:], ms[:], AF.Ln, bias=EPS, scale=float(1.0 / HD)
                        )
                        rec = psmall.tile([1, 512], f32, tag="small")
                        nc.scalar.activation(rec[:], lms[:], AF.Exp, scale=-0.5)
                        rstd = pbc.tile([128, 512], f32, tag="bc")
                        nc.gpsimd.partition_broadcast(rstd[:], rec[:])
                        nc.vector.tensor_tensor(t1[:], t1[:], rot[:], ALU.add)
                        nc.vector.tensor_tensor(
                            dst[:, tsl], t1[:], rstd[:], ALU.mult
                        )

                    # v projection first (attention j=c4 needs v chunks <= c4)
                    for tsub in range(4):
                        ps_v = ppsum_sm.tile([128, 256], f32, tag="sm", name="ps_v")
                        for kc in range(KC):
                            nc.tensor.matmul(
                                ps_v[:],
                                xts[kc // 2][:, kc % 2, tsub * 128 : tsub * 128 + 128],
                                wv[:, kc, :],
                                start=(kc == 0),
                                stop=(kc == KC - 1),
                            )
                        vi = c4 * 4 + tsub
                        for hh in range(2):
                            nc.vector.tensor_copy(
                                vt[hh][:, vi, :],
                                ps_v[:, hh * 128 : hh * 128 + 128],
                            )
                        if tsub == 0:
                            flush_pend()
                        advance(4)

                    # q/k projections + split norm/rope, one instance at a time
                    for qki in range(2):
                        for hh in range(2):
                            ps = ppsum_sm.tile(
                                [128, 512], f32, tag="sm", name=f"psqk_{qki}_{hh}"
                            )
                            col = (qki * 2 + hh) * 128
                            for kc in range(KC):
                                nc.tensor.matmul(
                                    ps[:],
                                    wqk[:, kc, col : col + 128],
                                    xts[kc // 2][:, kc % 2, :],
                                    start=(kc == 0),
                                    stop=(kc == KC - 1),
                                )
                            bidx = pair * 4 + qki * 2 + hh
                            flush_pend()
                            tsl = slice(c4 * 512, c4 * 512 + 512)
                            q_sb, sq, rot = norm_stageA(ps, bidx)
                            pend[0] = (
                                lambda q_sb=q_sb, sq=sq, rot=rot, qki=qki,
                                dst=qk[(qki, hh)], tsl=tsl: norm_stageB(
                                    q_sb, sq, rot, qki, dst, tsl
                                )
                            )
                            advance(3)

                    for hh in range(2):
                        gens.append((c4, gen_attn(pair, hh, c4, qk, vt, aTs[hh])))
            flush_pend()
            # ---- out projection, interleaved with the attention drain ----
            # stream w_out into the freed wv (heads 0,1) / wqk (heads 2,3)
            # slots, chunked along D so the first matmuls start early
            wo_a = pw.tile([128, 2, D], f32r, tag="wv", name="wo_a")
            wo_b = pw.tile([128, 2, D], f32r, tag="wqk", name="wo_b")
            for n4 in range(4):
                nsl = slice(n4 * 512, n4 * 512 + 512)
                nc.sync.dma_start(wo_a[:, :, nsl], wout_d[:, 0:2, nsl])
                nc.sync.dma_start(wo_b[:, :, nsl], wout_d[:, 2:4, nsl])
            wo = {0: wo_a, 1: wo_a, 2: wo_b, 3: wo_b}
            for tc_i in range(TK):
                # attnT writes for this t-chunk must be EMITTED before the
                # reads (Tile deps follow emission order)
                drain_until(tc_i // 4)
                tsl = slice(tc_i * 128, tc_i * 128 + 128)
                for n2 in range(D // 1024):
                    ps = ppsum.tile([128, 1024], f32, tag="big", name="ps_out")
                    for half in range(2):
                        nsl = slice(n2 * 1024 + half * 512, n2 * 1024 + half * 512 + 512)
                        for hi in range(4):
                            nc.tensor.matmul(
                                ps[:, half * 512 : half * 512 + 512],
                                attnT[(hi // 2, hi % 2)][:, tsl],
                                wo[hi][:, hi % 2, nsl],
                                start=(hi == 0),
                                stop=(hi == 3),
                            )
                    # past tc 12 the generators are fully drained, so the
                    # idle exp-tile slots can double the staging depth (the
                    # out DMA is the binding latency in the tail)
                    if tc_i >= 12 and (tc_i * 2 + n2) % 2 == 0:
                        o = pexp.tile([128, 1024], f32, tag="e", name="o_stage")
                    else:
                        o = px.tile([128, 1024], f32, tag="x", name="o_stage")
                    if (tc_i * 2 + n2) % 2 == 0:
                        nc.vector.tensor_copy(o[:], ps[:])
                    else:
                        nc.scalar.copy(o[:], ps[:])
                    nc.sync.dma_start(
                        out_d[tsl, n2 * 1024 : n2 * 1024 + 1024], o[:]
                    )
                    advance(2)
            drain()

    nc.compile()
    return nc


def host_inputs(x, w_qkv, b_qkv, q_scale, k_scale, w_out, L, share_cos=True):
    """Build the 8 per-core input maps."""
    x = np.asarray(x, np.float32)
    w_qkv = np.asarray(w_qkv, np.float32)
    b_qkv = np.asarray(b_qkv, np.float32)
    w_out = np.asarray(w_out, np.float32)
    q_scale = np.asarray(q_scale, np.float32)
    k_scale = np.asarray(k_scale, np.float32)

    half = HD // 2
    inv_freq = 1.0 / (ROPE_THETA ** (np.arange(half, dtype=np.float64) / half))
    pos = np.arange(L, dtype=np.float64)
    ang = pos[None, :] * inv_freq[:, None]          # [64, L]
    cos_t = np.cos(ang)
    sin_t = np.sin(ang)
    import ml_dtypes
    cosT = np.concatenate([cos_t, cos_t], 0)            # [128, L]
    sinT = np.concatenate([-sin_t, sin_t], 0)           # [-s; +s]
    # fold the qk-norm output scales into the rope tables: output dim i is
    # (pre-rope dim i)*cos_i + (pre-rope dim (i+64)%128)*sin_i, all scaled
    # by scale_i.
    def fold(scale):
        c = (cosT * scale[:, None]).astype(np.float32)
        s = (sinT * scale[:, None]).astype(np.float32)
        return np.ascontiguousarray(c), np.ascontiguousarray(s)

    qcosT, qsinT = fold(q_scale)
    kcosT, ksinT = fold(k_scale)

    # consolidated straddle mask: M[i, u] = 0 iff u >= i + 384 else NEG;
    # slice [384-128r : 896-128r] gives the r-straddle [128, 512] mask
    ii = np.arange(128)[:, None]
    uu = np.arange(896)[None, :]
    maskneg = np.ascontiguousarray(
        np.where(uu >= ii + 384, 0.0, NEG).astype(ml_dtypes.bfloat16)
    )

    onesv = np.ones((128, 1), np.float32)
    onesb = np.ones((128, 1), ml_dtypes.bfloat16)

    in_maps = []
    for c in range(N_CORES):
        b = c // 4
        g = c % 4
        heads = [4 * g + i for i in range(4)]
        xT = np.ascontiguousarray(x[b].T)                       # [D, L]
        xTr = np.ascontiguousarray(
            xT.reshape(KC, 128, L).transpose(1, 0, 2)
        )                                                        # [128, KC, L]
        wqk = np.empty((2, 128, KC, 512), np.float32)
        wv = np.empty((2, 128, KC, 256), np.float32)
        biasr = np.empty((128, 8), np.float32)   # [hd, (pair, qki, hh)]
        for p in range(2):
            hp = heads[2 * p : 2 * p + 2]
            cols = np.concatenate(
                [
                    np.arange(qki * D + h * HD, qki * D + (h + 1) * HD)
                    for qki in range(2)
                    for h in hp
                ]
            )
            wqk[p] = w_qkv[:, cols].reshape(KC, 128, 512).transpose(1, 0, 2)
            vcols = np.concatenate(
                [np.arange(2 * D + h * HD, 2 * D + (h + 1) * HD) for h in hp]
            )
            wv[p] = w_qkv[:, vcols].reshape(KC, 128, 256).transpose(1, 0, 2)
            for qki in range(2):
                for hh in range(2):
                    biasr[:, p * 4 + qki * 2 + hh] = b_qkv[
                        qki * D + hp[hh] * HD : qki * D + (hp[hh] + 1) * HD
                    ]

        wout = (
            w_out[heads[0] * HD : (heads[-1] + 1) * HD]
            .reshape(4, 128, D)
            .transpose(1, 0, 2)
        )
        in_maps.append(
            {
                "xT": np.ascontiguousarray(xTr),
                "wqk": np.ascontiguousarray(wqk),
                "wv": np.ascontiguousarray(wv),
                "wout": np.ascontiguousarray(wout),
                "qcosT": qcosT,
                "qsinT": qsinT,
                **({} if share_cos else {"kcosT": kcosT, "ksinT": ksinT}),
                "biasr": np.ascontiguousarray(biasr),
                "maskneg": maskneg,
                "ones": onesv,
                "ones_bf": onesb,
            }
        )
    return in_maps


_NC_CACHE = {}


def _get_nc(L, share_cos=True):
    key = (L, share_cos)
    if key not in _NC_CACHE:
        _NC_CACHE[key] = build_nc(L, share_cos=share_cos)
    return _NC_CACHE[key]


def run(x, w_qkv, b_qkv, q_scale, k_scale, w_out, b_out, L, **rb_kwargs):
    share_cos = bool(np.array_equal(np.asarray(q_scale), np.asarray(k_scale)))
    nc = _get_nc(L, share_cos)
    in_maps = host_inputs(x, w_qkv, b_qkv, q_scale, k_scale, w_out, L,
                          share_cos=share_cos)
    res = run_bass_kernel_spmd(nc, in_maps, list(range(N_CORES)), **rb_kwargs)
    parts = np.stack([r["out_p"] for r in res.results])          # [8, L, D]
    b_v = np.asarray(b_qkv, np.float64)[2 * D : 3 * D]
    bias_eff = np.asarray(b_out, np.float64) + b_v @ np.asarray(w_out, np.float64)
    out = np.empty((B, L, D), np.float32)
    for b in range(B):
        out[b] = parts[4 * b : 4 * b + 4].sum(0, dtype=np.float64) + bias_eff
    return out, res


def kernel(x, w_qkv, b_qkv, q_scale, k_scale, w_out, b_out, mask):
    out, _ = run(x, w_qkv, b_qkv, q_scale, k_scale, w_out, b_out, L=x.shape[1])
    return out



# revision 19
# speedup vs baseline: 1.0641x; 1.0292x over previous
"""Trainium2 Bass kernel for fused attention layer (QKV proj + QKNorm + RoPE +
causal attention + out proj), tensor-parallel across (batch, head-group) on 8
NeuronCores.

Reference semantics (B=2, L=2048, D=2048, H=16, HD=128):
    qkv = x @ w_qkv + b_qkv ; split q,k,v per head
    q,k = rms_norm(q)*q_scale, rms_norm(k)*k_scale   (over head_dim)
    q,k = rope(q), rope(k)                           (rotate-half)
    attn = softmax(mask(q k^T / sqrt(HD)))
    out = (attn @ v) reshaped @ w_out + b_out
Sharding: core c -> batch b=c//4, heads 4g..4g+3 with g=c%4. Each core emits a
partial out-projection [L, D]; the host sums the 4 partials per batch + b_out.

v2 scheduling notes:
  - qkv bias is applied by a 17th 1-partition matmul in each accumulation
    group (stationary = bias row, moving = a [1,512] ones vector), so the
    norm/rope pipeline reads the biased projection directly from PSUM.
  - q_scale/k_scale are folded into per-q/k cos/sin tables on the host
    (rope output dim i scales by s_i, which multiplies cos_i and the sin
    factor sourced from partition (i+64)%128).
  - DMA issue order puts the first-consumed bytes (wv chunk 0, x tiles)
    ahead of wqk/constants so the PE starts ~3us in.
  - w_out streams into the freed wv/wqk slots right after the last
    projection reads, chunked so the first out-proj matmuls start early.
  - out projection DMAs straight from PSUM (no staging copy).
"""

import numpy as np

import concourse.bass as bass
import concourse.bacc as bacc
import concourse.tile as tile
import concourse.mybir as mybir
from concourse.bass_utils import run_bass_kernel_spmd

# This kernel only uses Square/Ln/Exp/Copy/Identity activations, all present
# in the 'natural_log_exp_and_others' set. The greedy per-function set
# chooser would otherwise ping-pong between 'exp_and_others' and
# 'natural_log' (a ~1.3us table load per switch, on the softmax critical
# path). Restrict candidacy to the one set that covers everything; entry
# order (= act_func_set_id) is preserved so walrus still loads the right
# table.
_orig_get_act_tables = bacc.get_activation_tables


def _act_tables_one_set(arch):
    tables = _orig_get_act_tables(arch)
    keep = "natural_log_exp_and_others"
    if keep in tables:
        tables = {
            name: (funcs if name == keep else type(funcs)())
            for name, funcs in tables.items()
        }
    return tables


bacc.get_activation_tables = _act_tables_one_set

f32 = mybir.dt.float32
f16 = mybir.dt.float16
f32r = mybir.dt.float32r
bf16 = mybir.dt.bfloat16
AF = mybir.ActivationFunctionType
ALU = mybir.AluOpType

B = 2
D = 2048
H = 16
HD = 128
KC = D // 128          # 16 contraction chunks of 128
EPS = 1e-6
ROPE_THETA = 10000.0
NEG = -1e30
N_CORES = 8
HEADS_PER_CORE = 4     # 2 pairs of 2


def build_nc(L, share_cos=True):
    TQ = L // 512       # 512-wide t chunks (query chunks, proj chunks)
    TK = L // 128       # 128-wide t chunks (key chunks, v chunks, out-proj chunks)

    nc = bacc.Bacc(None, target_bir_lowering=False)

    xT_d = nc.dram_tensor("xT", [128, KC, L], f32r, kind="ExternalInput")
    wqk_d = nc.dram_tensor("wqk", [2, 128, KC, 512], f32r, kind="ExternalInput")
    wv_d = nc.dram_tensor("wv", [2, 128, KC, 256], f32r, kind="ExternalInput")
    wout_d = nc.dram_tensor("wout", [128, 4, D], f32r, kind="ExternalInput")
    qcos_d = nc.dram_tensor("qcosT", [128, L], f32, kind="ExternalInput")
    qsin_d = nc.dram_tensor("qsinT", [128, L], f32, kind="ExternalInput")
    if not share_cos:
        kcos_d = nc.dram_tensor("kcosT", [128, L], f32, kind="ExternalInput")
        ksin_d = nc.dram_tensor("ksinT", [128, L], f32, kind="ExternalInput")
    biasr_d = nc.dram_tensor("biasr", [128, 8], f32, kind="ExternalInput")
    mask_d = nc.dram_tensor("maskneg", [128, 896], bf16, kind="ExternalInput")
    ones_d = nc.dram_tensor("ones", [128, 1], f32r, kind="ExternalInput")
    onesb_d = nc.dram_tensor("ones_bf", [128, 1], bf16, kind="ExternalInput")
    out_d = nc.dram_tensor("out_p", [L, D], f32, kind="ExternalOutput")

    inv_sqrt_hd = float(1.0 / np.sqrt(HD))
    # exp(score - EXP_BIAS): keeps e strictly inside f16 range for any input
    # (|score| <= sqrt(HD) = 11.32 after QKNorm, exp(11.32-1.5) = 1.8e4);
    # the constant cancels exactly in the rowsum normalization.
    EXP_BIAS = -1.5

    # register float consts used as activation biases
    for cval in (EPS, 0.0, EXP_BIAS):
        if (f32, cval) in nc.const_aps.aps:
            continue
        _t = nc.alloc_sbuf_tensor(f"const-float32-{cval}", [128, 1], f32)
        nc.gpsimd.memset(_t.ap(), cval)
        nc.const_aps.aps[(f32, cval)] = _t.ap()
    nc.all_engine_barrier()

    from contextlib import ExitStack

    with ExitStack() as ctx:
        tc = ctx.enter_context(tile.TileContext(nc))
        ctx.enter_context(
            nc.allow_low_precision(
                reason="f32r rounding of matmul operands is intentional"
            )
        )
        px = ctx.enter_context(tc.tile_pool(name="px", bufs=8))      # x stream
        pw = ctx.enter_context(tc.tile_pool(name="pw", bufs=1))       # wqk + wv
        pqk = ctx.enter_context(tc.tile_pool(name="pqk", bufs=4))     # qT/kT
        pv = ctx.enter_context(tc.tile_pool(name="pv", bufs=1))       # v slab
        pat = ctx.enter_context(tc.tile_pool(name="pat", bufs=4))     # attn_outT
        ptab = ctx.enter_context(tc.tile_pool(name="ptab", bufs=1))   # constants
        pscr = ctx.enter_context(tc.tile_pool(name="pscr", bufs=1))   # scratch
        pexp = ctx.enter_context(tc.tile_pool(name="pexp", bufs=2))   # exp tiles
        pbc = ctx.enter_context(tc.tile_pool(name="pbc", bufs=2))     # broadcasts
        psmall = ctx.enter_context(tc.tile_pool(name="psmall", bufs=2))
        ppsum = ctx.enter_context(tc.tile_pool(name="psum", bufs=3, space="PSUM"))
        ppsum_sm = ctx.enter_context(tc.tile_pool(name="psum_sm", bufs=2, space="PSUM"))
        if True:
            # ---- constant tiles (DMAs issued later, in startup-aware order)
            qcos = ptab.tile([128, L], f32, tag="qcos")
            qsin = ptab.tile([128, L], f32, tag="qsin")
            if share_cos:
                kcos, ksin = qcos, qsin
            else:
                kcos = ptab.tile([128, L], f32, tag="kcos")
                ksin = ptab.tile([128, L], f32, tag="ksin")
            biasr = ptab.tile([128, 8], f32, tag="biasr")
            masks = ptab.tile([128, 896], bf16, tag="masks")
            ones = ptab.tile([128, 1], f32r, tag="ones")
            onesb = ptab.tile([128, 1], bf16, tag="onesb")

            def dma_small_consts():
                nc.sync.dma_start(masks[:], mask_d[:])
                nc.sync.dma_start(biasr[:], biasr_d[:])
                nc.sync.dma_start(ones[:], ones_d[:])
                nc.sync.dma_start(onesb[:], onesb_d[:])

            def dma_cos_consts():
                nc.sync.dma_start(qcos[:], qcos_d[:])
                nc.sync.dma_start(qsin[:], qsin_d[:])
                if not share_cos:
                    nc.sync.dma_start(kcos[:], kcos_d[:])
                    nc.sync.dma_start(ksin[:], ksin_d[:])

            attnT = {}  # (pair, hh) -> [128, L] f32r

            def gen_attn(pair, hh, j, qk, vt, aT):
                """Generator emitting one (head, q-chunk) attention unit,
                yielding after each 2-block group so projection work can be
                interleaved between groups."""
                qT = qk[(0, hh)]
                kT = qk[(1, hh)]
                jsl = slice(j * 512, j * 512 + 512)
                jhi = slice(j * 512 + 256, j * 512 + 512)
                ncc = 4 * (j + 1)
                nfull = ncc - 2     # last two diagonal chunks run half-width
                ps_o_t = ppsum.tile([128, 1024], f32, tag="big", name="ps_o")
                ps_o = ps_o_t[:, 0:512]
                ps_sum = ps_o_t[0:1, 512:1024]
                # software-pipelined: scores+exp for group g are emitted one
                # iteration ahead of g's av/rowsum, so the PE runs group
                # g+1's scores while the scalar engine exps group g
                pending_av = None   # (e_tile, cp) awaiting av/rowsum

                def emit_av(e, cp, width, qoff):
                    for ci in range(2):
                        c = cp + ci
                        esl = e[:, ci * width : ci * width + width]
                        nc.tensor.matmul(
                            ps_sum[:, qoff : qoff + width], ones[:], esl,
                            start=(c == 0), stop=(c == ncc - 1),
                            skip_group_check=True,
                        )
                        nc.tensor.matmul(
                            ps_o[:, qoff : qoff + width], vt[hh][:, c, :], esl,
                            start=(c == 0), stop=(c == ncc - 1),
                            skip_group_check=True,
                        )

                for cp in range(0, nfull, 2):
                    ps_s = ppsum.tile([128, 1024], f32, tag="big", name="ps_s")
                    for ci in range(2):
                        c = cp + ci
                        nc.tensor.matmul(
                            ps_s[:, ci * 512 : ci * 512 + 512],
                            kT[:, c * 128 : c * 128 + 128],
                            qT[:, jsl],
                            start=True,
                            stop=True,
                        )
                        r = c - 4 * j
                        if r >= 0:
                            ms0 = 384 - 128 * r
                            nc.vector.tensor_tensor(
                                ps_s[:, ci * 512 : ci * 512 + 512],
                                ps_s[:, ci * 512 : ci * 512 + 512],
                                masks[:, ms0 : ms0 + 512],
                                ALU.add,
                            )
                    e = pexp.tile([128, 1024], f32r, tag="e")
                    for ci in range(2):
                        nc.scalar.activation(
                            e[:, ci * 512 : ci * 512 + 512],
                            ps_s[:, ci * 512 : ci * 512 + 512],
                            AF.Exp, scale=inv_sqrt_hd, bias=EXP_BIAS,
                        )
                    if pending_av is not None:
                        emit_av(*pending_av)
                    pending_av = (e, cp, 512, 0)
                    yield
                # diagonal pair (r=2,3): q columns [0,256) of the upper half
                # are fully masked, so only compute the top 256 q columns
                ps_s = ppsum.tile([128, 1024], f32, tag="big", name="ps_s")
                for ci in range(2):
                    c = nfull + ci
                    nc.tensor.matmul(
                        ps_s[:, ci * 256 : ci * 256 + 256],
                        kT[:, c * 128 : c * 128 + 128],
                        qT[:, jhi],
                        start=True,
                        stop=True,
                    )
                    # valid iff q >= k: within this [128, 256] tile the
                    # allowed region is u' >= i + 128*(ci-1) + 128
                    ms0 = 384 - 128 * ci
                    nc.vector.tensor_tensor(
                        ps_s[:, ci * 256 : ci * 256 + 256],
                        ps_s[:, ci * 256 : ci * 256 + 256],
                        masks[:, ms0 : ms0 + 256],
                        ALU.add,
                    )
                e = pexp.tile([128, 1024], f32r, tag="e")
                nc.scalar.activation(
                    e[:, 0:512], ps_s[:, 0:512], AF.Exp, scale=inv_sqrt_hd,
                    bias=EXP_BIAS,
                )
                if pending_av is not None:
                    emit_av(*pending_av)
                emit_av(e, nfull, 256, 256)
                yield
                # normalize straight out of PSUM (fused copy+scale)
                reca = psmall.tile([1, 512], f32, tag="small")
                nc.vector.reciprocal_approx_fast(reca[:], ps_sum)
                rbc = pbc.tile([128, 512], f32, tag="bc")
                nc.gpsimd.partition_broadcast(rbc[:], reca[:])
                nc.vector.tensor_tensor(aT[:, jsl], ps_o, rbc[:], ALU.mult)

            from collections import deque

            gens = deque()

            def advance(n):
                for _ in range(n):
                    while gens:
                        try:
                            next(gens[0][1])
                            break
                        except StopIteration:
                            gens.popleft()
                    if not gens:
                        break

            def drain():
                while gens:
                    try:
                        next(gens[0][1])
                    except StopIteration:
                        gens.popleft()

            def drain_until(j_max):
                # fully drain every pending generator labelled <= j_max
                # (FIFO front is always oldest; labels are per-pair ascending)
                while any(lbl <= j_max for lbl, _ in gens):
                    try:
                        next(gens[0][1])
                    except StopIteration:
                        gens.popleft()

            for pair in range(2):
                # ---- weights for this pair (wv first: v-proj runs first) ----
                wqk = pw.tile([128, KC, 512], f32r, tag="wqk")
                wv = pw.tile([128, KC, 256], f32r, tag="wv")
                nc.sync.dma_start(wv[:, 0 : KC // 4], wv_d[pair][:, 0 : KC // 4])
                if pair != 0:
                    for kq in range(1, 4):
                        ksl = slice(kq * (KC // 4), (kq + 1) * (KC // 4))
                        nc.sync.dma_start(wv[:, ksl], wv_d[pair][:, ksl])
                    for kq in range(4):
                        ksl = slice(kq * (KC // 4), (kq + 1) * (KC // 4))
                        nc.sync.dma_start(wqk[:, ksl], wqk_d[pair][:, ksl])

                if pair != 0:
                    # pair 0's pending attention reads the v/qk slabs this
                    # pair is about to overwrite: drain the OLDEST units so
                    # emission order stays write-after-read safe; the rest
                    # drains during this pair's projections
                    drain_until(3)
                vt = [
                    pv.tile([128, TK, 128], f32r, tag=f"v{hh}", name=f"v_{hh}")
                    for hh in range(2)
                ]
                qk = {}
                for qki in range(2):
                    for hh in range(2):
                        qk[(qki, hh)] = pqk.tile(
                            [128, L], f32r, tag="qk", name=f"qk_{qki}_{hh}"
                        )
                aTs = {}
                for hh in range(2):
                    aTs[hh] = pat.tile([128, L], f32r, tag="attnT", name=f"aT_{hh}")
                    attnT[(pair, hh)] = aTs[hh]

                # ---- projection + norm + rope + interleaved attention ----
                for c4 in range(TQ):
                    t0 = c4 * 512
                    xts = []
                    for kh in range(8):
                        xt = px.tile([128, 2, 512], f32r, tag="x", name=f"xt_{kh}")
                        nc.sync.dma_start(
                            xt[:], xT_d[:, 2 * kh : 2 * kh + 2, t0 : t0 + 512]
                        )
                        xts.append(xt)
                    if pair == 0 and c4 == 0:
                        # startup: x + wv chunk 0 already queued; rest follows
                        for kq in range(1, 4):
                            ksl = slice(kq * (KC // 4), (kq + 1) * (KC // 4))
                            nc.sync.dma_start(wv[:, ksl], wv_d[pair][:, ksl])
                        dma_small_consts()
                        for kq in range(4):
                            ksl = slice(kq * (KC // 4), (kq + 1) * (KC // 4))
                            nc.sync.dma_start(wqk[:, ksl], wqk_d[pair][:, ksl])
                        dma_cos_consts()
                    # fill the x-DMA window with pending attention blocks
                    advance(14)

                    def norm_stageA(ps, bidx):
                        # squares for the rms-norm sum: ACT reads PSUM
                        # directly and adds the qkv bias per partition
                        sq = pscr.tile([128, 512], bf16, tag="sq")
                        nc.scalar.activation(
                            sq[:], ps[:], AF.Square,
                            bias=biasr[:, bidx : bidx + 1],
                        )
                        # biased projection staged to SBUF for the rope
                        # cos path + rotate-half DMA (DMA can't read PSUM)
                        q_sb = pscr.tile([128, 512], f32, tag="q_sb")
                        nc.vector.tensor_scalar_add(
                            q_sb[:], ps[:], biasr[:, bidx : bidx + 1]
                        )
                        rot = pscr.tile([128, 512], f32, tag="rot")
                        nc.sync.dma_start(rot[0:64, :], q_sb[64:128, :])
                        nc.sync.dma_start(rot[64:128, :], q_sb[0:64, :])
                        return q_sb, sq, rot

                    def norm_stageB(q_sb, sq, rot, qki, hh, tsl):
                        cosT = qcos if qki == 0 else kcos
                        sinT = qsin if qki == 0 else ksin
                        t1 = pscr.tile([128, 512], f32, tag="t1")
                        nc.vector.tensor_tensor(t1[:], q_sb[:], cosT[:, tsl], ALU.mult)
                        nc.vector.tensor_tensor(rot[:], rot[:], sinT[:, tsl], ALU.mult)
                        # deferred: the ms matmul runs once ACT's square is
                        # long done, so PE never stalls on it
                        ms = ppsum_sm.tile([1, 512], f32, tag="sm", name="ms")
                        nc.tensor.matmul(
                            ms[:], onesb[:], sq[:], start=True, stop=True
                        )
                        # rstd = exp(-0.5*ln(ms/HD + eps)): Ln and Exp share
                        # one activation-function set with Square/Copy, so
                        # the scalar engine never reloads tables (Sqrt would)
                        lms = psmall.tile([1, 512], f32, tag="small")
                        nc.scalar.activation(
                            lms[:], ms[:], AF.Ln, bias=EPS, scale=float(1.0 / HD)
                        )
                        rec = psmall.tile([1, 512], f32, tag="small")
                        nc.scalar.activation(rec[:], lms[:], AF.Exp, scale=-0.5)
                        rstd = pbc.tile([128, 512], f32, tag="bc")
                        nc.gpsimd.partition_broadcast(rstd[:], rec[:])
                        nc.vector.tensor_tensor(t1[:], t1[:], rot[:], ALU.add)
                        nc.vector.tensor_tensor(
                            qk[(qki, hh)][:, tsl], t1[:], rstd[:], ALU.mult
                        )

                    pending_B = None

                    # v projection first (attention j=c4 needs v chunks <= c4)
                    for tsub in range(4):
                        ps_v = ppsum_sm.tile([128, 256], f32, tag="sm", name="ps_v")
                        for kc in range(KC):
                            nc.tensor.matmul(
                                ps_v[:],
                                xts[kc // 2][:, kc % 2, tsub * 128 : tsub * 128 + 128],
                                wv[:, kc, :],
                                start=(kc == 0),
                                stop=(kc == KC - 1),
                            )
                        vi = c4 * 4 + tsub
                        for hh in range(2):
                            nc.vector.tensor_copy(
                                vt[hh][:, vi, :],
                                ps_v[:, hh * 128 : hh * 128 + 128],
                            )
                        advance(2)

                    # q/k projections + split norm/rope, one instance at a time
                    for qki in range(2):
                        for hh in range(2):
                            ps = ppsum_sm.tile(
                                [128, 512], f32, tag="sm", name=f"psqk_{qki}_{hh}"
                            )
                            col = (qki * 2 + hh) * 128
                            for kc in range(KC):
                                nc.tensor.matmul(
                                    ps[:],
                                    wqk[:, kc, col : col + 128],
                                    xts[kc // 2][:, kc % 2, :],
                                    start=(kc == 0),
                                    stop=(kc == KC - 1),
                                )
                            bidx = pair * 4 + qki * 2 + hh
                            if pending_B is not None:
                                pending_B()
                            tsl = slice(c4 * 512, c4 * 512 + 512)
                            q_sb, sq, rot = norm_stageA(ps, bidx)
                            pending_B = (
                                lambda q_sb=q_sb, sq=sq, rot=rot, qki=qki,
                                hh=hh, tsl=tsl: norm_stageB(
                                    q_sb, sq, rot, qki, hh, tsl
                                )
                            )
                            advance(2)
                    pending_B()
                    pending_B = None

                    for hh in range(2):
                        gens.append((c4, gen_attn(pair, hh, c4, qk, vt, aTs[hh])))
            # ---- out projection, interleaved with the attention drain ----
            # stream w_out into the freed wv (heads 0,1) / wqk (heads 2,3)
            # slots, chunked along D so the first matmuls start early
            wo_a = pw.tile([128, 2, D], f32r, tag="wv", name="wo_a")
            wo_b = pw.tile([128, 2, D], f32r, tag="wqk", name="wo_b")
            for n4 in range(4):
                nsl = slice(n4 * 512, n4 * 512 + 512)
                nc.sync.dma_start(wo_a[:, :, nsl], wout_d[:, 0:2, nsl])
                nc.sync.dma_start(wo_b[:, :, nsl], wout_d[:, 2:4, nsl])
            wo = {0: wo_a, 1: wo_a, 2: wo_b, 3: wo_b}
            for tc_i in range(TK):
                # attnT writes for this t-chunk must be EMITTED before the
                # reads (Tile deps follow emission order)
                drain_until(tc_i // 4)
                tsl = slice(tc_i * 128, tc_i * 128 + 128)
                for n2 in range(D // 1024):
                    ps = ppsum.tile([128, 1024], f32, tag="big", name="ps_out")
                    for half in range(2):
                        nsl = slice(n2 * 1024 + half * 512, n2 * 1024 + half * 512 + 512)
                        for hi in range(4):
                            nc.tensor.matmul(
                                ps[:, half * 512 : half * 512 + 512],
                                attnT[(hi // 2, hi % 2)][:, tsl],
                                wo[hi][:, hi % 2, nsl],
                                start=(hi == 0),
                                stop=(hi == 3),
                            )
                    # past tc 12 the generators are fully drained, so the
                    # idle exp-tile slots can double the staging depth (the
                    # out DMA is the binding latency in the tail)
                    if tc_i >= 12 and (tc_i * 2 + n2) % 2 == 0:
                        o = pexp.tile([128, 1024], f32, tag="e", name="o_stage")
                    else:
                        o = px.tile([128, 1024], f32, tag="x", name="o_stage")
                    if (tc_i * 2 + n2) % 2 == 0:
                        nc.vector.tensor_copy(o[:], ps[:])
                    else:
                        nc.scalar.copy(o[:], ps[:])
                    nc.sync.dma_start(
                        out_d[tsl, n2 * 1024 : n2 * 1024 + 1024], o[:]
                    )
                    advance(2)
            drain()

    nc.compile()
    return nc


def host_inputs(x, w_qkv, b_qkv, q_scale, k_scale, w_out, L, share_cos=True):
    """Build the 8 per-core input maps."""
    x = np.asarray(x, np.float32)
    w_qkv = np.asarray(w_qkv, np.float32)
    b_qkv = np.asarray(b_qkv, np.float32)
    w_out = np.asarray(w_out, np.float32)
    q_scale = np.asarray(q_scale, np.float32)
    k_scale = np.asarray(k_scale, np.float32)

    half = HD // 2
    inv_freq = 1.0 / (ROPE_THETA ** (np.arange(half, dtype=np.float64) / half))
    pos = np.arange(L, dtype=np.float64)
    ang = pos[None, :] * inv_freq[:, None]          # [64, L]
    cos_t = np.cos(ang)
    sin_t = np.sin(ang)
    import ml_dtypes
    cosT = np.concatenate([cos_t, cos_t], 0)            # [128, L]
    sinT = np.concatenate([-sin_t, sin_t], 0)           # [-s; +s]
    # fold the qk-norm output scales into the rope tables: output dim i is
    # (pre-rope dim i)*cos_i + (pre-rope dim (i+64)%128)*sin_i, all scaled
    # by scale_i.
    def fold(scale):
        c = (cosT * scale[:, None]).astype(np.float32)
        s = (sinT * scale[:, None]).astype(np.float32)
        return np.ascontiguousarray(c), np.ascontiguousarray(s)

    qcosT, qsinT = fold(q_scale)
    kcosT, ksinT = fold(k_scale)

    # consolidated straddle mask: M[i, u] = 0 iff u >= i + 384 else NEG;
    # slice [384-128r : 896-128r] gives the r-straddle [128, 512] mask
    ii = np.arange(128)[:, None]
    uu = np.arange(896)[None, :]
    maskneg = np.ascontiguousarray(
        np.where(uu >= ii + 384, 0.0, NEG).astype(ml_dtypes.bfloat16)
    )

    onesv = np.ones((128, 1), np.float32)
    onesb = np.ones((128, 1), ml_dtypes.bfloat16)

    in_maps = []
    for c in range(N_CORES):
        b = c // 4
        g = c % 4
        heads = [4 * g + i for i in range(4)]
        xT = np.ascontiguousarray(x[b].T)                       # [D, L]
        xTr = np.ascontiguousarray(
            xT.reshape(KC, 128, L).transpose(1, 0, 2)
        )                                                        # [128, KC, L]
        wqk = np.empty((2, 128, KC, 512), np.float32)
        wv = np.empty((2, 128, KC, 256), np.float32)
        biasr = np.empty((128, 8), np.float32)   # [hd, (pair, qki, hh)]
        for p in range(2):
            hp = heads[2 * p : 2 * p + 2]
            cols = np.concatenate(
                [
                    np.arange(qki * D + h * HD, qki * D + (h + 1) * HD)
                    for qki in range(2)
                    for h in hp
                ]
            )
            wqk[p] = w_qkv[:, cols].reshape(KC, 128, 512).transpose(1, 0, 2)
            vcols = np.concatenate(
                [np.arange(2 * D + h * HD, 2 * D + (h + 1) * HD) for h in hp]
            )
            wv[p] = w_qkv[:, vcols].reshape(KC, 128, 256).transpose(1, 0, 2)
            for qki in range(2):
                for hh in range(2):
                    biasr[:, p * 4 + qki * 2 + hh] = b_qkv[
                        qki * D + hp[hh] * HD : qki * D + (hp[hh] + 1) * HD
                    ]

        wout = (
            w_out[heads[0] * HD : (heads[-1] + 1) * HD]
            .reshape(4, 128, D)
            .transpose(1, 0, 2)
        )
        in_maps.append(
            {
                "xT": np.ascontiguousarray(xTr),
                "wqk": np.ascontiguousarray(wqk),
                "wv": np.ascontiguousarray(wv),
                "wout": np.ascontiguousarray(wout),
                "qcosT": qcosT,
                "qsinT": qsinT,
                **({} if share_cos else {"kcosT": kcosT, "ksinT": ksinT}),
                "biasr": np.ascontiguousarray(biasr),
                "maskneg": maskneg,
                "ones": onesv,
                "ones_bf": onesb,
            }
        )
    return in_maps


_NC_CACHE = {}


def _get_nc(L, share_cos=True):
    key = (L, share_cos)
    if key not in _NC_CACHE:
        _NC_CACHE[key] = build_nc(L, share_cos=share_cos)
    return _NC_CACHE[key]


def run(x, w_qkv, b_qkv, q_scale, k_scale, w_out, b_out, L, **rb_kwargs):
    share_cos = bool(np.array_equal(np.asarray(q_scale), np.asarray(k_scale)))
    nc = _get_nc(L, share_cos)
    in_maps = host_inputs(x, w_qkv, b_qkv, q_scale, k_scale, w_out, L,
                          share_cos=share_cos)
    res = run_bass_kernel_spmd(nc, in_maps, list(range(N_CORES)), **rb_kwargs)
    parts = np.stack([r["out_p"] for r in res.results])          # [8, L, D]
    b_v = np.asarray(b_qkv, np.float64)[2 * D : 3 * D]
    bias_eff = np.asarray(b_out, np.float64) + b_v @ np.asarray(w_out, np.float64)
    out = np.empty((B, L, D), np.float32)
    for b in range(B):
        out[b] = parts[4 * b : 4 * b + 4].sum(0, dtype=np.float64) + bias_eff
    return out, res


def kernel(x, w_qkv, b_qkv, q_scale, k_scale, w_out, b_out, mask):
    out, _ = run(x, w_qkv, b_qkv, q_scale, k_scale, w_out, b_out, L=x.shape[1])
    return out

